# revision 23
# baseline (speedup 1.0000x reference)
"""BasesDecomposition (R-GCN style) message passing kernel for Trainium2.

Strategy (8 NeuronCores, SPMD — one program, per-core data):
  - Nodes sharded by row: core c owns targets [c*NL, (c+1)*NL).
  - Edges symmetrized on host, partitioned by target-owner core, grouped
    by relation (padded per relation to a cross-core-uniform multiple of
    128 so the chunk -> W_r schedule is program-static); within a
    relation, edges are ordered by target block so each (rel, blk) run
    is contiguous in message space.
  - Per-relation weights W_r = sum_b rbw[r, b] * bases[b] (host, bf16).
  - Host builds, per core, a pre-transposed pre-scaled source table
    xsT[d, pos] = x[src_e, d] * ew_e (bf16, zeros at padding), so
    phase 1 is a plain sequential DMA load (no gather, no transpose).
  - Phase 1: per 128-edge chunk, one bf16 matmul msg = xsT_chunk^T @ W_r
    (PSUM f32), cast-copied (vector/scalar alternate) into an 8-chunk
    staging tile, then one DMA writes the 1024 message rows to md (bf16,
    row-major) in DRAM.
  - Phase 2 (aggregate, one 128-target block per iteration): one
    indirect gather pulls SL consecutive md rows for each of 128
    interval starts covering the block's per-relation runs; a single
    wide tensor_tensor(is_equal) against an iota constant builds the
    0/1 scatter matrix T for all SL slices at once (edge weights were
    folded into xsT, invalid slots hold -1); SL accumulating matmuls
    out^T += mg_j^T @ T_j plus a self-loop matmul W_self^T @ xmt_blk
    run in PSUM; result block is written to outT.
  - Host reassembles out from the per-core outT blocks.
"""

import numpy as np
import ml_dtypes

import concourse.bass as bass
import concourse.bacc as bacc
import concourse.tile as tile
import concourse.mybir as mybir
from concourse.bass_utils import run_bass_kernel_spmd

F32 = mybir.dt.float32
BF16 = mybir.dt.bfloat16
I32 = mybir.dt.int32

NCORE = 8
R = 32           # num_relations (relation id R is the self-loop row of rbw)
LDTOK = 4096     # tokens per xsT load slice
MDG = 16         # chunks per md write group
PS4 = 4          # matmul outputs packed per PSUM bank (one wide copy each)
SL_CANDIDATES = (8, 10, 12, 16)
BF = ml_dtypes.bfloat16


def _splits(total, step):
    out = []
    off = 0
    while off < total:
        sz = min(step, total - off)
        out.append((off, sz))
        off += sz
    return out


def host_prep(x, node_keep_mask, source, target, edge_type, edge_weights,
              bases, relation_base_weights):
    n, d = x.shape
    assert n % NCORE == 0
    nl = n // NCORE
    nblk = (nl + 127) // 128
    nlp = nblk * 128

    f32 = np.float32
    W = np.einsum("rb,bdo->rdo", relation_base_weights.astype(f32),
                  bases.astype(f32)).astype(f32)  # (R+1, 128, 128)
    wsb_h = np.ascontiguousarray(
        W.transpose(1, 0, 2).reshape(d, (R + 1) * d)).astype(BF)

    src2 = np.concatenate([source, target]).astype(np.int64)
    tgt2 = np.concatenate([target, source]).astype(np.int64)
    et2 = np.concatenate([edge_type, edge_type]).astype(np.int64)
    ew2 = np.concatenate([edge_weights, edge_weights]).astype(f32)

    owner = tgt2 // nl
    tloc = tgt2 - owner * nl
    blk = tloc // 128
    tin = (tloc - blk * 128).astype(f32)

    # per-(core, rel) counts; per-relation group size uniform across cores.
    # Rounded to 256 so md can be written in partition-paired order (each
    # SBUF partition supplies 2 consecutive md rows -> 512B descriptors).
    cr = owner * R + et2
    cnt = np.bincount(cr, minlength=NCORE * R).reshape(NCORE, R)
    Gr = ((cnt.max(axis=0) + 255) // 256) * 256          # (R,)
    starts_r = np.concatenate([[0], np.cumsum(Gr)[:-1]])  # (R,)
    ep1 = int(Gr.sum())
    nch = ep1 // 128
    rel_of_chunk = np.repeat(np.arange(R), Gr // 128)

    # rank within (core, rel), ordered by target block
    starts_cr = np.concatenate([[0], np.cumsum(cnt.reshape(-1))[:-1]])
    order = np.lexsort((blk, cr))
    rank = np.empty(len(cr), np.int64)
    rank[order] = np.arange(len(cr)) - starts_cr[cr[order]]
    pos = starts_r[et2] + rank  # core-local md row of each edge

    # xsT column of md row m: within each 256-row md group, chunk j (=m%2)
    # holds rows {2p+j} on partition p, so the phase-1 md write emits 512B
    # per-partition descriptors. perm: m -> g*256 + (m%2)*128 + (m%256)//2
    def md_to_xst(m):
        g = m // 256
        r = m % 256
        return g * 256 + (r % 2) * 128 + r // 2

    # per-(core, rel, blk) run lengths and starts within the rel group
    crb = cr * nblk + blk
    cnt_crb = np.bincount(crb, minlength=NCORE * R * nblk).reshape(
        NCORE, R, nblk)
    run_start = np.zeros_like(cnt_crb)
    run_start[:, :, 1:] = np.cumsum(cnt_crb, axis=2)[:, :, :-1]

    # smallest interval stride whose per-(core, block) interval count fits
    # in the 128 indices of one indirect gather
    for SL in SL_CANDIDATES:
        n_iv = np.ceil(cnt_crb / SL).sum(axis=1).max()
        if n_iv <= 128:
            break
    else:
        raise AssertionError(f"no SL fits: {n_iv} intervals")

    xf = x.astype(f32)
    keep = node_keep_mask.astype(f32)

    per_core = []
    for c in range(NCORE):
        m = owner == c
        vals = xf[src2[m]] * ew2[m, None]
        xsT_h = np.zeros((128, ep1), BF)
        xsT_h[:, md_to_xst(pos[m])] = vals.T.astype(BF)

        # md row -> (block, tin) of the edge it holds (-1 if padding)
        row_blk = np.full(ep1, -1, np.int64)
        row_tin = np.full(ep1, -1.0, f32)
        row_blk[pos[m]] = blk[m]
        row_tin[pos[m]] = tin[m]

        # phase-2 cover: per block, interval starts covering the runs
        cidx_h = np.zeros((128, nblk), np.int32)
        tcol_h = np.full((128, nblk, SL), -1.0, f32)
        for b in range(nblk):
            p = 0
            for r in range(R):
                s = int(starts_r[r] + run_start[c, r, b])
                ln = int(cnt_crb[c, r, b])
                for off in range(0, ln, SL):
                    st = min(s + off, ep1 - SL)
                    lo = s + off
                    hi = min(s + off + SL, s + ln)
                    assert p < 128, "cover overflow"
                    cidx_h[p, b] = st
                    sl_rows = np.arange(st, st + SL)
                    use = (sl_rows >= lo) & (sl_rows < hi)
                    tcol_h[p, b, use] = row_tin[sl_rows[use]]
                    p += 1
        tcol_h = np.ascontiguousarray(
            tcol_h.transpose(0, 1, 2).reshape(128, nblk * SL)).astype(BF)

        xm = xf[c * nl:(c + 1) * nl] * keep[c * nl:(c + 1) * nl, None]
        xmt_h = np.zeros((128, nlp), BF)
        xmt_h[:, :nl] = xm.T.astype(BF)

        per_core.append({
            "xsT": xsT_h,
            "wsb": wsb_h,
            "xmt": xmt_h,
            "cidx": np.ascontiguousarray(cidx_h),
            "tcol": tcol_h,
        })

    cfg = dict(n=n, nl=nl, nblk=nblk, nlp=nlp, ep1=ep1, nch=nch, SL=SL,
               rel_of_chunk=tuple(int(r) for r in rel_of_chunk))
    return per_core, cfg


def build_program(cfg):
    n = cfg["n"]
    nblk = cfg["nblk"]
    nlp = cfg["nlp"]
    ep1 = cfg["ep1"]
    SL = cfg["SL"]
    rel_of_chunk = cfg["rel_of_chunk"]

    nc = bacc.Bacc(None, target_bir_lowering=False, debug=False)

    xsT = nc.declare_dram_parameter("xsT", [128, ep1], BF16, isOutput=False)
    wsb = nc.declare_dram_parameter("wsb", [128, (R + 1) * 128], BF16,
                                    isOutput=False)
    xmt = nc.declare_dram_parameter("xmt", [128, nlp], BF16, isOutput=False)
    cidx = nc.declare_dram_parameter("cidx", [128, nblk], I32, isOutput=False)
    tcol = nc.declare_dram_parameter("tcol", [128, nblk * SL], BF16,
                                     isOutput=False)
    outT = nc.declare_dram_parameter("outT", [128, nlp], F32, isOutput=True)

    md = nc.dram_tensor("md", [ep1, 128], BF16)

    colw_d = nc.inline_tensor(
        np.tile(np.arange(128, dtype=np.float32), (128, 2 * SL)), name="colw_c")

    with tile.TileContext(nc) as tc:
        with tc.tile_pool(name="const", bufs=1) as constp:
            wsb_t = constp.tile([128, (R + 1) * 128], BF16)
            nc.sync.dma_start(out=wsb_t[:], in_=wsb[:])
            xmt_t = constp.tile([128, nlp], BF16)
            nc.sync.dma_start(out=xmt_t[:], in_=xmt[:])
            cidx_t = constp.tile([128, nblk], I32)
            nc.sync.dma_start(out=cidx_t[:], in_=cidx[:])
            tcol_t = constp.tile([128, nblk * SL], BF16)
            nc.sync.dma_start(out=tcol_t[:], in_=tcol[:])
            colw_f = constp.tile([128, 2 * SL * 128], F32)
            nc.sync.dma_start(out=colw_f[:], in_=colw_d[:])
            colw = constp.tile([128, 2 * SL * 128], BF16)
            nc.vector.tensor_copy(out=colw[:], in_=colw_f[:])

            # ---------------- Phase 1: messages ----------------
            with (
                tc.tile_pool(name="xt", bufs=3) as xp,
                tc.tile_pool(name="mds", bufs=3) as mdp,
                tc.tile_pool(name="p1ps", bufs=8, space="PSUM") as p1ps,
            ):
                ncopy = 0
                for loff, lsz in _splits(ep1, LDTOK):
                    xt = xp.tile([128, LDTOK], BF16, tag="xt")
                    nc.sync.dma_start(out=xt[:, :lsz],
                                      in_=xsT[:, loff:loff + lsz])
                    for goff, gsz in _splits(lsz, MDG * 128):
                        ms = mdp.tile([128, MDG * 128], BF16, tag="ms")
                        for poff, psz in _splits(gsz, PS4 * 128):
                            ps = p1ps.tile([128, PS4 * 128], F32, tag="ps")
                            for j in range(psz // 128):
                                ch = (loff + goff + poff) // 128 + j
                                r = rel_of_chunk[ch]
                                nc.tensor.matmul(
                                    out=ps[:, 128 * j:128 * (j + 1)],
                                    lhsT=xt[:, goff + poff + 128 * j:
                                            goff + poff + 128 * (j + 1)],
                                    rhs=wsb_t[:, 128 * r:128 * (r + 1)],
                                    start=True, stop=True)
                            ncopy += 1
                            if ncopy % 2 == 0:
                                nc.vector.tensor_copy(
                                    out=ms[:, poff:poff + psz],
                                    in_=ps[:, :psz])
                            else:
                                nc.scalar.copy(
                                    out=ms[:, poff:poff + psz],
                                    in_=ps[:, :psz])
                        c0 = (loff + goff) // 128
                        mdv = md[c0 * 128:(c0 + gsz // 128) * 128, :]
                        nc.sync.dma_start(
                            out=mdv.rearrange("(g p j) c -> p g (j c)",
                                              p=128, j=2),
                            in_=ms[:, :gsz].rearrange("p (g w) -> p g w",
                                                      w=256))

            # ---------------- Phase 2: aggregate ----------------
            with (
                tc.tile_pool(name="p2", bufs=6) as p2,
                tc.tile_pool(name="p2t", bufs=4) as p2t,
                tc.tile_pool(name="io", bufs=4) as iop,
                tc.tile_pool(name="p2ps", bufs=4, space="PSUM") as p2ps,
            ):
                assert nblk % 2 == 0
                for b0 in range(0, nblk, 2):
                    # one wide T build serves two blocks; gathers stay
                    # per-block (HW ignores extra offset columns)
                    tt = p2t.tile([128, 2 * SL * 128], BF16, tag="T")
                    tsl = tcol_t[:, b0 * SL:(b0 + 2) * SL]
                    tb = tsl.unsqueeze(2).broadcast_to([128, 2 * SL, 128])
                    nc.vector.tensor_tensor(
                        out=tt[:].rearrange("p (s e) -> p s e", e=128),
                        in0=colw[:].rearrange("p (s e) -> p s e", e=128),
                        in1=tb, op=mybir.AluOpType.is_equal)
                    for k in range(2):
                        b = b0 + k
                        mg = p2.tile([128, SL * 128], BF16, tag="mg")
                        nc.gpsimd.indirect_dma_start(
                            out=mg[:], out_offset=None, in_=md[:, :],
                            in_offset=bass.IndirectOffsetOnAxis(
                                ap=cidx_t[:, b:b + 1], axis=0))
                        ps = p2ps.tile([128, 128], F32, tag="acc")
                        for j in range(SL):
                            nc.tensor.matmul(
                                out=ps[:],
                                lhsT=mg[:, 128 * j:128 * (j + 1)],
                                rhs=tt[:, 128 * (k * SL + j):
                                       128 * (k * SL + j + 1)],
                                start=(j == 0), stop=False)
                        nc.tensor.matmul(
                            out=ps[:],
                            lhsT=wsb_t[:, R * 128:(R + 1) * 128],
                            rhs=xmt_t[:, 128 * b:128 * (b + 1)],
                            start=False, stop=True)
                        ob = iop.tile([128, 128], F32, tag="ob")
                        nc.scalar.copy(out=ob[:], in_=ps[:])
                        nc.sync.dma_start(out=outT[:, 128 * b:128 * (b + 1)],
                                          in_=ob[:])

    nc.finalize()
    return nc


_PROGRAM_CACHE = {}


def _get_program(cfg):
    key = tuple(sorted((k, v) for k, v in cfg.items()))
    if key not in _PROGRAM_CACHE:
        _PROGRAM_CACHE[key] = build_program(cfg)
    return _PROGRAM_CACHE[key]


def kernel(x, node_keep_mask, source, target, edge_type, edge_weights,
           bases, relation_base_weights):
    per_core, cfg = host_prep(x, node_keep_mask, source, target, edge_type,
                              edge_weights, bases, relation_base_weights)
    nc = _get_program(cfg)
    res = run_bass_kernel_spmd(nc, per_core, list(range(NCORE)))
    nl = cfg["nl"]
    out = np.empty((cfg["n"], 128), np.float32)
    for c in range(NCORE):
        out[c * nl:(c + 1) * nl] = res.results[c]["outT"][:, :nl].T
    return out


# revision 25
# speedup vs baseline: 1.0971x; 1.0971x over previous
"""BasesDecomposition (R-GCN style) message passing kernel for Trainium2.

Strategy (8 NeuronCores, SPMD — one program, per-core data):
  - Nodes sharded by row: core c owns targets [c*NL, (c+1)*NL).
  - Edges symmetrized on host, partitioned by target-owner core, grouped
    by relation (padded per relation to a cross-core-uniform multiple of
    128 so the chunk -> W_r schedule is program-static); within a
    relation, edges are ordered by target block so each (rel, blk) run
    is contiguous in message space.
  - Per-relation weights W_r = sum_b rbw[r, b] * bases[b] (host, bf16).
  - Host builds, per core, a pre-transposed pre-scaled source table
    xsT[d, pos] = x[src_e, d] * ew_e (bf16, zeros at padding), so
    phase 1 is a plain sequential DMA load (no gather, no transpose).
  - Phase 1: per 128-edge chunk, one bf16 matmul msg = xsT_chunk^T @ W_r
    (PSUM f32), cast-copied (vector/scalar alternate) into an 8-chunk
    staging tile, then one DMA writes the 1024 message rows to md (bf16,
    row-major) in DRAM.
  - Phase 2 (aggregate, one 128-target block per iteration): one
    indirect gather pulls SL consecutive md rows for each of 128
    interval starts covering the block's per-relation runs; a single
    wide tensor_tensor(is_equal) against an iota constant builds the
    0/1 scatter matrix T for all SL slices at once (edge weights were
    folded into xsT, invalid slots hold -1); SL accumulating matmuls
    out^T += mg_j^T @ T_j plus a self-loop matmul W_self^T @ xmt_blk
    run in PSUM; result block is written to outT.
  - Host reassembles out from the per-core outT blocks.
"""

import numpy as np
import ml_dtypes

import concourse.bass as bass
import concourse.bacc as bacc
import concourse.tile as tile
import concourse.mybir as mybir
from concourse.bass_utils import run_bass_kernel_spmd

F32 = mybir.dt.float32
BF16 = mybir.dt.bfloat16
I32 = mybir.dt.int32

NCORE = 8
R = 32           # num_relations (relation id R is the self-loop row of rbw)
LDTOK = 4096     # tokens per xsT load slice
MDG = 16         # chunks per md write group
PS4 = 4          # matmul outputs packed per PSUM bank (one wide copy each)
SL_CANDIDATES = (8, 10, 12, 16)
BF = ml_dtypes.bfloat16


def _splits(total, step):
    out = []
    off = 0
    while off < total:
        sz = min(step, total - off)
        out.append((off, sz))
        off += sz
    return out


def host_prep(x, node_keep_mask, source, target, edge_type, edge_weights,
              bases, relation_base_weights):
    n, d = x.shape
    assert n % NCORE == 0
    nl = n // NCORE
    nblk = (nl + 127) // 128
    nlp = nblk * 128

    f32 = np.float32
    W = np.einsum("rb,bdo->rdo", relation_base_weights.astype(f32),
                  bases.astype(f32)).astype(f32)  # (R+1, 128, 128)
    wsb_h = np.ascontiguousarray(
        W.transpose(1, 0, 2).reshape(d, (R + 1) * d)).astype(BF)

    src2 = np.concatenate([source, target]).astype(np.int64)
    tgt2 = np.concatenate([target, source]).astype(np.int64)
    et2 = np.concatenate([edge_type, edge_type]).astype(np.int64)
    ew2 = np.concatenate([edge_weights, edge_weights]).astype(f32)

    owner = tgt2 // nl
    tloc = tgt2 - owner * nl
    blk = tloc // 128
    tin = (tloc - blk * 128).astype(f32)

    # per-(core, rel) counts; per-relation group size uniform across cores.
    # Rounded to 256 so md can be written in partition-paired order (each
    # SBUF partition supplies 2 consecutive md rows -> 512B descriptors).
    cr = owner * R + et2
    cnt = np.bincount(cr, minlength=NCORE * R).reshape(NCORE, R)
    Gr = ((cnt.max(axis=0) + 255) // 256) * 256          # (R,)
    starts_r = np.concatenate([[0], np.cumsum(Gr)[:-1]])  # (R,)
    ep1 = int(Gr.sum())
    nch = ep1 // 128
    rel_of_chunk = np.repeat(np.arange(R), Gr // 128)

    # rank within (core, rel), ordered by target block
    starts_cr = np.concatenate([[0], np.cumsum(cnt.reshape(-1))[:-1]])
    order = np.lexsort((blk, cr))
    rank = np.empty(len(cr), np.int64)
    rank[order] = np.arange(len(cr)) - starts_cr[cr[order]]
    pos = starts_r[et2] + rank  # core-local md row of each edge

    # xsT column of md row m: within each 256-row md group, chunk j (=m%2)
    # holds rows {2p+j} on partition p, so the phase-1 md write emits 512B
    # per-partition descriptors. perm: m -> g*256 + (m%2)*128 + (m%256)//2
    def md_to_xst(m):
        g = m // 256
        r = m % 256
        return g * 256 + (r % 2) * 128 + r // 2

    # per-(core, rel, blk) run lengths and starts within the rel group
    crb = cr * nblk + blk
    cnt_crb = np.bincount(crb, minlength=NCORE * R * nblk).reshape(
        NCORE, R, nblk)
    run_start = np.zeros_like(cnt_crb)
    run_start[:, :, 1:] = np.cumsum(cnt_crb, axis=2)[:, :, :-1]

    # smallest interval stride whose per-(core, block) interval count fits
    # in the 128 indices of one indirect gather
    for SL in SL_CANDIDATES:
        n_iv = np.ceil(cnt_crb / SL).sum(axis=1).max()
        if n_iv <= 128:
            break
    else:
        raise AssertionError(f"no SL fits: {n_iv} intervals")

    xf = x.astype(f32)
    keep = node_keep_mask.astype(f32)

    per_core = []
    for c in range(NCORE):
        m = owner == c
        vals = xf[src2[m]] * ew2[m, None]
        xsT_h = np.zeros((128, ep1), BF)
        xsT_h[:, md_to_xst(pos[m])] = vals.T.astype(BF)

        # md row -> (block, tin) of the edge it holds (-1 if padding)
        row_blk = np.full(ep1, -1, np.int64)
        row_tin = np.full(ep1, -1.0, f32)
        row_blk[pos[m]] = blk[m]
        row_tin[pos[m]] = tin[m]

        # phase-2 cover: per block, interval starts covering the runs
        cidx_h = np.zeros((128, nblk), np.int32)
        tcol_h = np.full((128, nblk, SL), -1.0, f32)
        for b in range(nblk):
            p = 0
            for r in range(R):
                s = int(starts_r[r] + run_start[c, r, b])
                ln = int(cnt_crb[c, r, b])
                for off in range(0, ln, SL):
                    st = min(s + off, ep1 - SL)
                    lo = s + off
                    hi = min(s + off + SL, s + ln)
                    assert p < 128, "cover overflow"
                    cidx_h[p, b] = st
                    sl_rows = np.arange(st, st + SL)
                    use = (sl_rows >= lo) & (sl_rows < hi)
                    tcol_h[p, b, use] = row_tin[sl_rows[use]]
                    p += 1
        tcol_h = np.ascontiguousarray(
            tcol_h.transpose(0, 1, 2).reshape(128, nblk * SL)).astype(BF)

        xm = xf[c * nl:(c + 1) * nl] * keep[c * nl:(c + 1) * nl, None]
        xmt_h = np.zeros((128, nlp), BF)
        xmt_h[:, :nl] = xm.T.astype(BF)

        per_core.append({
            "xsT": xsT_h,
            "wsb": wsb_h,
            "xmt": xmt_h,
            "cidx": np.ascontiguousarray(cidx_h),
            "tcol": tcol_h,
        })

    cfg = dict(n=n, nl=nl, nblk=nblk, nlp=nlp, ep1=ep1, nch=nch, SL=SL,
               rel_of_chunk=tuple(int(r) for r in rel_of_chunk))
    return per_core, cfg


def build_program(cfg):
    n = cfg["n"]
    nblk = cfg["nblk"]
    nlp = cfg["nlp"]
    ep1 = cfg["ep1"]
    SL = cfg["SL"]
    rel_of_chunk = cfg["rel_of_chunk"]

    nc = bacc.Bacc(None, target_bir_lowering=False, debug=False)

    xsT = nc.declare_dram_parameter("xsT", [128, ep1], BF16, isOutput=False)
    wsb = nc.declare_dram_parameter("wsb", [128, (R + 1) * 128], BF16,
                                    isOutput=False)
    xmt = nc.declare_dram_parameter("xmt", [128, nlp], BF16, isOutput=False)
    cidx = nc.declare_dram_parameter("cidx", [128, nblk], I32, isOutput=False)
    tcol = nc.declare_dram_parameter("tcol", [128, nblk * SL], BF16,
                                     isOutput=False)
    outT = nc.declare_dram_parameter("outT", [128, nlp], F32, isOutput=True)

    md = nc.dram_tensor("md", [ep1, 128], BF16)

    colw_d = nc.inline_tensor(
        np.tile(np.arange(128, dtype=np.float32), (128, 2 * SL)), name="colw_c")

    with tile.TileContext(nc) as tc:
        with tc.tile_pool(name="const", bufs=1) as constp:
            wsb_t = constp.tile([128, (R + 1) * 128], BF16)
            nc.sync.dma_start(out=wsb_t[:], in_=wsb[:])
            xmt_t = constp.tile([128, nlp], BF16)
            nc.sync.dma_start(out=xmt_t[:], in_=xmt[:])
            cidx_t = constp.tile([128, nblk], I32)
            nc.sync.dma_start(out=cidx_t[:], in_=cidx[:])
            tcol_t = constp.tile([128, nblk * SL], BF16)
            nc.sync.dma_start(out=tcol_t[:], in_=tcol[:])
            colw_f = constp.tile([128, 2 * SL * 128], F32)
            nc.sync.dma_start(out=colw_f[:], in_=colw_d[:])
            colw = constp.tile([128, 2 * SL * 128], BF16)
            nc.vector.tensor_copy(out=colw[:], in_=colw_f[:])

            # ---------------- Phase 1: messages ----------------
            with (
                tc.tile_pool(name="xt", bufs=3) as xp,
                tc.tile_pool(name="mds", bufs=3) as mdp,
                tc.tile_pool(name="p1ps", bufs=8, space="PSUM") as p1ps,
            ):
                ncopy = 0
                for loff, lsz in _splits(ep1, LDTOK):
                    xt = xp.tile([128, LDTOK], BF16, tag="xt")
                    nc.sync.dma_start(out=xt[:, :lsz],
                                      in_=xsT[:, loff:loff + lsz])
                    for goff, gsz in _splits(lsz, MDG * 128):
                        ms = mdp.tile([128, MDG * 128], BF16, tag="ms")
                        for poff, psz in _splits(gsz, PS4 * 128):
                            ps = p1ps.tile([128, PS4 * 128], F32, tag="ps")
                            for j in range(psz // 128):
                                ch = (loff + goff + poff) // 128 + j
                                r = rel_of_chunk[ch]
                                nc.tensor.matmul(
                                    out=ps[:, 128 * j:128 * (j + 1)],
                                    lhsT=xt[:, goff + poff + 128 * j:
                                            goff + poff + 128 * (j + 1)],
                                    rhs=wsb_t[:, 128 * r:128 * (r + 1)],
                                    start=True, stop=True)
                            ncopy += 1
                            if ncopy % 2 == 0:
                                nc.vector.tensor_copy(
                                    out=ms[:, poff:poff + psz],
                                    in_=ps[:, :psz])
                            else:
                                nc.scalar.copy(
                                    out=ms[:, poff:poff + psz],
                                    in_=ps[:, :psz])
                        c0 = (loff + goff) // 128
                        mdv = md[c0 * 128:(c0 + gsz // 128) * 128, :]
                        nc.sync.dma_start(
                            out=mdv.rearrange("(g p j) c -> p g (j c)",
                                              p=128, j=2),
                            in_=ms[:, :gsz].rearrange("p (g w) -> p g w",
                                                      w=256))

            # ---------------- Phase 2: aggregate ----------------
            with (
                tc.tile_pool(name="p2", bufs=8) as p2,
                tc.tile_pool(name="p2t", bufs=6) as p2t,
                tc.tile_pool(name="io", bufs=4) as iop,
                tc.tile_pool(name="p2ps", bufs=4, space="PSUM") as p2ps,
            ):
                for b in range(nblk):
                    mg = p2.tile([128, SL * 128], BF16, tag="mg")
                    nc.gpsimd.indirect_dma_start(
                        out=mg[:], out_offset=None, in_=md[:, :],
                        in_offset=bass.IndirectOffsetOnAxis(
                            ap=cidx_t[:, b:b + 1], axis=0))
                    tt = p2t.tile([128, SL * 128], BF16, tag="T")
                    tsl = tcol_t[:, b * SL:(b + 1) * SL]
                    tb = tsl.unsqueeze(2).broadcast_to([128, SL, 128])
                    nc.vector.tensor_tensor(
                        out=tt[:].rearrange("p (s e) -> p s e", e=128),
                        in0=colw[:, :SL * 128].rearrange("p (s e) -> p s e",
                                                         e=128),
                        in1=tb, op=mybir.AluOpType.is_equal)
                    ps = p2ps.tile([128, 128], F32, tag="acc")
                    for j in range(SL):
                        nc.tensor.matmul(
                            out=ps[:],
                            lhsT=mg[:, 128 * j:128 * (j + 1)],
                            rhs=tt[:, 128 * j:128 * (j + 1)],
                            start=(j == 0), stop=False)
                    nc.tensor.matmul(
                        out=ps[:],
                        lhsT=wsb_t[:, R * 128:(R + 1) * 128],
                        rhs=xmt_t[:, 128 * b:128 * (b + 1)],
                        start=False, stop=True)
                    ob = iop.tile([128, 128], F32, tag="ob")
                    nc.scalar.copy(out=ob[:], in_=ps[:])
                    nc.sync.dma_start(out=outT[:, 128 * b:128 * (b + 1)],
                                      in_=ob[:])

    nc.finalize()
    return nc


_PROGRAM_CACHE = {}


def _get_program(cfg):
    key = tuple(sorted((k, v) for k, v in cfg.items()))
    if key not in _PROGRAM_CACHE:
        _PROGRAM_CACHE[key] = build_program(cfg)
    return _PROGRAM_CACHE[key]


def kernel(x, node_keep_mask, source, target, edge_type, edge_weights,
           bases, relation_base_weights):
    per_core, cfg = host_prep(x, node_keep_mask, source, target, edge_type,
                              edge_weights, bases, relation_base_weights)
    nc = _get_program(cfg)
    res = run_bass_kernel_spmd(nc, per_core, list(range(NCORE)))
    nl = cfg["nl"]
    out = np.empty((cfg["n"], 128), np.float32)
    for c in range(NCORE):
        out[c * nl:(c + 1) * nl] = res.results[c]["outT"][:, :nl].T
    return out
